# revision 15
# baseline (speedup 1.0000x reference)
"""Causal self-attention Trainium2 kernel.

Sharding: 8 cores = (4 batches) x (2 head-groups of 8 heads).
Each core: projections for its 512 channels, causal attention for its 8
heads over its batch, partial out-projection over its 512 channels.
Host: sums the two partials per batch and adds the output bias.

Layouts on core (b = fixed batch, channels o in [0,512) local):
  xT   [128f, 8fc, 2048t]  - x transposed (PE transpose), streamed per 512-t chunk
  qT/kT [128o, 4oc, 2048t] - head h = oc*2+hh occupies partitions hh*64..+64 of chunk oc
  vx   [128t, 16tj, 8h*65] - v natural + a ones column per head (softmax denominator)
  scores^T tiles [128j, 512i] -> exp on ACT (scale=1/8) -> masked (staircase input)
  attn^T accum psum [65, 512i] rows 0..63 = head out, row 64 = denom
  out  psum [128t, 512c] -> DMA straight to DRAM partial
"""

from contextlib import ExitStack

import numpy as np

import concourse.bass as bass
import concourse.mybir as mybir
import concourse.tile as tile
from concourse.masks import make_identity

P = 128
C = 1024  # d_model
CL = 512  # local channels (8 heads * 64)
D = 64  # head dim
NH = 8  # local heads
FC = C // P  # 8 f-chunks
OC = CL // P  # 4 o-chunks
F32 = mybir.dt.float32
F32R = mybir.dt.float32r
AF = mybir.ActivationFunctionType
GROUP = 3  # score jt-tiles per exp call (3 psum banks, double buffered)


def _emit(nc, tc, ctx, T):
    NT = T // P  # 128-token chunks
    T4 = T // 512  # 512-token chunks

    xb = nc.dram_tensor("xb", [T, C], F32, kind="ExternalInput")
    wq_d = nc.dram_tensor("wq", [C, CL], F32R, kind="ExternalInput")
    wk_d = nc.dram_tensor("wk", [C, CL], F32R, kind="ExternalInput")
    wv_d = nc.dram_tensor("wv", [C, CL], F32R, kind="ExternalInput")
    wo_d = nc.dram_tensor("wo", [CL, C], F32R, kind="ExternalInput")
    bq_d = nc.dram_tensor("bq", [CL], F32, kind="ExternalInput")
    bk_d = nc.dram_tensor("bk", [CL], F32, kind="ExternalInput")
    bv_d = nc.dram_tensor("bv", [CL], F32R, kind="ExternalInput")
    stair_d = nc.dram_tensor("stair", [P, 1024], F32, kind="ExternalInput")
    outp = nc.dram_tensor("outp", [T, C], F32, kind="ExternalOutput")

    const = ctx.enter_context(tc.tile_pool(name="const", bufs=1))
    ident = const.tile([P, P], F32)
    make_identity(nc, ident)
    ones1f = const.tile([1, P], F32)
    nc.gpsimd.memset(ones1f[:], 1.0)
    ones1 = const.tile([1, P], F32R)
    nc.vector.tensor_copy(ones1[:], ones1f[:])
    bq_sb = const.tile([P, OC], F32)
    nc.sync.dma_start(bq_sb[:], bq_d.rearrange("(oc p) -> p oc", p=P))
    bk_sb = const.tile([P, OC], F32)
    nc.sync.dma_start(bk_sb[:], bk_d.rearrange("(oc p) -> p oc", p=P))
    bv_sb = const.tile([1, CL], F32R)
    nc.sync.dma_start(bv_sb[:], bv_d.rearrange("(a c) -> a c", a=1))
    stair_sb = const.tile([P, 1024], F32)
    nc.sync.dma_start(stair_sb[:], stair_d[:])

    qkv = ctx.enter_context(tc.tile_pool(name="qkv", bufs=1))
    qT = qkv.tile([P, OC, T], F32R)
    kT = qkv.tile([P, OC, T], F32R)
    vx = qkv.tile([P, NT, NH * 65], F32R)
    vx5 = vx.rearrange("p n (h u) -> p n h u", u=65)
    onesv = const.tile([P, NT * NH], F32)
    nc.gpsimd.memset(onesv[:], 1.0)
    nc.vector.tensor_copy(
        vx5[:, :, :, 64:65], onesv[:].rearrange("p (n h) -> p n h", h=NH)[:, :, :, None]
    )

    # ---------------- projections ----------------
    with (
        tc.tile_pool(name="wpool", bufs=1) as wpool,
        tc.tile_pool(name="xin_pool", bufs=2) as xin_pool,
        tc.tile_pool(name="xT_pool", bufs=1) as xT_pool,
        tc.tile_pool(name="tp_ps", bufs=4, space="PSUM") as tp_ps,
        tc.tile_pool(name="pj_ps", bufs=3, space="PSUM") as pj_ps,
    ):
        wq_sb = wpool.tile([P, FC, CL], F32R)
        nc.sync.dma_start(wq_sb[:], wq_d.rearrange("(fc p) o -> p fc o", p=P))
        wk_sb = wpool.tile([P, FC, CL], F32R)
        nc.sync.dma_start(wk_sb[:], wk_d.rearrange("(fc p) o -> p fc o", p=P))
        wv_sb = wpool.tile([P, FC, CL], F32R)
        nc.sync.dma_start(wv_sb[:], wv_d.rearrange("(fc p) o -> p fc o", p=P))

        for tt in range(T4):
            xT = xT_pool.tile([P, FC, 512], F32R)
            for s in range(4):
                xin = xin_pool.tile([P, C], F32)
                nc.sync.dma_start(
                    xin[:], xb[(tt * 4 + s) * P : (tt * 4 + s + 1) * P, :]
                )
                for fc in range(FC):
                    tp = tp_ps.tile([P, P], F32)
                    nc.tensor.transpose(tp[:], xin[:, fc * P : (fc + 1) * P], ident[:])
                    nc.vector.tensor_copy(xT[:, fc, s * P : (s + 1) * P], tp[:])
            for oc in range(OC):
                for w_sb, b_sb, dT in ((wq_sb, bq_sb, qT), (wk_sb, bk_sb, kT)):
                    ps = pj_ps.tile([P, 512], F32, tag="pj")
                    for fc in range(FC):
                        nc.tensor.matmul(
                            ps[:],
                            w_sb[:, fc, oc * P : (oc + 1) * P],
                            xT[:, fc, :],
                            start=(fc == 0),
                            stop=(fc == FC - 1),
                        )
                    nc.vector.tensor_scalar_add(
                        dT[:, oc, tt * 512 : (tt + 1) * 512],
                        ps[:],
                        b_sb[:, oc : oc + 1],
                    )
            for s in range(4):
                ps = pj_ps.tile([P, 512], F32, tag="pj")
                for fc in range(FC):
                    nc.tensor.matmul(
                        ps[:],
                        xT[:, fc, s * P : (s + 1) * P],
                        wv_sb[:, fc, :],
                        start=(fc == 0),
                        stop=False,
                    )
                nc.tensor.matmul(
                    ps[:],
                    ones1[:],
                    bv_sb[:],
                    start=False,
                    stop=True,
                )
                nc.vector.tensor_copy(
                    vx5[:, tt * 4 + s, :, 0:64],
                    ps[:].rearrange("p (h d) -> p h d", d=D),
                )

    # ---------------- attention + out-projection ----------------
    wo_pool = ctx.enter_context(tc.tile_pool(name="wo_pool", bufs=1))
    attT_pool = ctx.enter_context(tc.tile_pool(name="attT_pool", bufs=1))
    wo_sb = wo_pool.tile([P, OC, C], F32R)
    nc.sync.dma_start(wo_sb[:], wo_d.rearrange("(oc p) c -> p oc c", p=P))
    attT = attT_pool.tile([P, OC, T], F32R)

    with (
        tc.tile_pool(name="exp_pool", bufs=3) as exp_pool,
        tc.tile_pool(name="nrm", bufs=2) as nrm_pool,
        tc.tile_pool(name="sc_ps", bufs=2, space="PSUM") as sc_ps_pool,
        tc.tile_pool(name="at_ps", bufs=2, space="PSUM") as at_ps_pool,
    ):
        for oc in range(OC):
            for hh in range(2):
                h = oc * 2 + hh
                base = hh * 64
                for ic in range(T4):
                    njt = ic * 4 + 4
                    at = at_ps_pool.tile([P, 512], F32)
                    for g0 in range(0, njt, GROUP):
                        grp = list(range(g0, min(g0 + GROUP, njt)))
                        n = len(grp)
                        sc = sc_ps_pool.tile([P, GROUP, 512], F32)
                        for si, jt in enumerate(grp):
                            nc.tensor.matmul(
                                sc[:, si, :],
                                kT[base : base + D, oc, jt * P : (jt + 1) * P],
                                qT[
                                    base : base + D, oc, ic * 512 : (ic + 1) * 512
                                ],
                                start=True,
                                stop=True,
                            )
                        ex = exp_pool.tile([P, GROUP, 512], F32R)
                        nc.scalar.activation(
                            ex[:, 0:n, :], sc[:, 0:n, :], AF.Exp, scale=0.125
                        )
                        for si, jt in enumerate(grp):
                            d = jt - ic * 4
                            if d >= 0:
                                w = (d + 1) * P
                                nc.vector.tensor_mul(
                                    ex[:, si, 0:w],
                                    ex[:, si, 0:w],
                                    stair_sb[:, 512 - d * P : 512 - d * P + w],
                                )
                        for si, jt in enumerate(grp):
                            nc.tensor.matmul(
                                at[0:65, :],
                                vx5[:, jt, h, :],
                                ex[:, si, :],
                                start=(jt == 0),
                                stop=(jt == njt - 1),
                            )
                    rc = nrm_pool.tile([1, 512], F32, tag="rc")
                    nc.vector.reciprocal(rc[:], at[64:65, :])
                    # recip row broadcast into psum rows 64..127 (K=1 outer product)
                    nc.tensor.matmul(
                        at[64:128, :], ones1f[:, 0:64], rc[:], start=True, stop=True
                    )
                    tmp = nrm_pool.tile([64, 512], F32, tag="tmp")
                    nc.vector.tensor_copy(tmp[:], at[0:64, :])
                    nc.vector.tensor_mul(
                        attT[base : base + D, oc, ic * 512 : (ic + 1) * 512],
                        tmp[:],
                        at[64:128, :],
                    )

    with (
        tc.tile_pool(name="op_ps", bufs=4, space="PSUM") as op_ps,
        tc.tile_pool(name="ob_pool", bufs=4) as ob_pool,
    ):
        for s16 in range(NT):
            for ch in range(2):
                ps = op_ps.tile([P, 512], F32)
                for oc in range(OC):
                    nc.tensor.matmul(
                        ps[:],
                        attT[:, oc, s16 * P : (s16 + 1) * P],
                        wo_sb[:, oc, ch * 512 : (ch + 1) * 512],
                        start=(oc == 0),
                        stop=(oc == OC - 1),
                    )
                ob = ob_pool.tile([P, 512], F32)
                nc.scalar.copy(ob[:], ps[:])
                nc.sync.dma_start(
                    outp[s16 * P : (s16 + 1) * P, ch * 512 : (ch + 1) * 512],
                    ob[:],
                )


def build(T=2048):
    nc = bass.Bass()
    with tile.TileContext(nc) as tc:
        with ExitStack() as ctx:
            _emit(nc, tc, ctx, T)
    return nc


def make_stair():
    j = np.arange(P)[:, None]
    u = np.arange(1024)[None, :]
    return (u >= j + 512).astype(np.float32)


def make_in_maps(x, wq, bq, wk, bk, wv, bv, wo):
    stair = make_stair()
    in_maps = []
    for c in range(8):
        b, g = c // 2, c % 2
        sl = slice(g * CL, (g + 1) * CL)
        in_maps.append(
            {
                "xb": np.ascontiguousarray(x[b]),
                "wq": np.ascontiguousarray(wq[:, sl]),
                "wk": np.ascontiguousarray(wk[:, sl]),
                "wv": np.ascontiguousarray(wv[:, sl]),
                "wo": np.ascontiguousarray(wo[sl, :]),
                "bq": np.ascontiguousarray(bq[sl]),
                "bk": np.ascontiguousarray(bk[sl]),
                "bv": np.ascontiguousarray(bv[sl]),
                "stair": stair,
            }
        )
    return in_maps


_cache = {}


def _split_multi_waits(bir_json: bytes) -> bytes:
    """Split instructions carrying >1 sync waits into single-wait NoOp
    chains on the same engine queue.  The TPB instruction encoding has one
    wait slot; this walrus build refuses multi-wait instructions instead
    of splitting them itself."""
    import orjson

    m = orjson.loads(bir_json)
    n = 0
    for fn in m.get("functions", []):
        for blk in fn.get("blocks", []):
            out = []
            for inst in blk.get("instructions", []):
                si = inst.get("sync_info")
                waits = si.get("on_wait") if si else None
                if waits and len(waits) > 1:
                    for w in waits[:-1]:
                        n += 1
                        out.append(
                            {
                                "debug": inst.get("debug", {}),
                                "engine": inst["engine"],
                                "ins": [],
                                "outs": [],
                                "name": f"{inst['name']}_sw{n}",
                                "opcode": "NoOp",
                                "text_hint": "split_wait",
                                "sync_info": {"on_wait": [w], "on_update": []},
                            }
                        )
                    si["on_wait"] = [waits[-1]]
                out.append(inst)
            blk["instructions"] = out
    return orjson.dumps(m)


def _install_compile_patch():
    import concourse.bass_utils as bu

    if getattr(bu, "_split_waits_patched", False):
        return
    orig = bu.compile_bir_kernel

    def patched(bir_json, tmpdir, neff_name="file.neff"):
        return orig(_split_multi_waits(bir_json), tmpdir, neff_name)

    bu.compile_bir_kernel = patched
    bu._split_waits_patched = True
    try:
        import concourse.bass2jax as b2j

        b2j.compile_bir_kernel = patched
    except ImportError:
        pass


def kernel(x, wq, bq, wk, bk, wv, bv, wo, bo):
    from concourse.bass_utils import run_bass_kernel_spmd

    _install_compile_patch()

    x = np.asarray(x, np.float32)
    args = [np.asarray(a, np.float32) for a in (wq, bq, wk, bk, wv, bv, wo, bo)]
    wq, bq, wk, bk, wv, bv, wo, bo = args
    B, T, _ = x.shape

    if "nc" not in _cache:
        _cache["nc"] = build(T)
    nc = _cache["nc"]

    in_maps = make_in_maps(x, wq, bq, wk, bk, wv, bv, wo)
    res = run_bass_kernel_spmd(nc, in_maps, core_ids=list(range(8)))
    out = np.empty((B, T, C), np.float32)
    for b in range(B):
        out[b] = res.results[2 * b]["outp"] + res.results[2 * b + 1]["outp"] + bo
    return out
